# revision 1
# baseline (speedup 1.0000x reference)
"""GAT 2-layer kernel for trn2: host prep (int-only) + Bass program builder.

Sharding: dst-node sharding across NCORES cores with global node relabeling
for load balance. Per-edge source rows fetched via dma_gather from replicated
bf16 tables in HBM; segment softmax/sum via one-hot selection matmuls on PE.
"""
import numpy as np
import ml_dtypes
from dataclasses import dataclass
from contextlib import ExitStack

from concourse import bass, mybir, tile, bacc
from concourse import library_config

P = 128
F32 = mybir.dt.float32
BF16 = mybir.dt.bfloat16
I16 = mybir.dt.int16
AF = mybir.ActivationFunctionType
ALU = mybir.AluOpType
BF = ml_dtypes.bfloat16


@dataclass
class Cfg:
    N: int = 50000
    E: int = 800000
    IN: int = 128
    HID: int = 16
    HEADS: int = 8
    NCORES: int = 8
    CBLK: int = 3          # blocks per gather chunk
    NBLK: int = 0
    PC: int = 0
    NP: int = 0
    F0: int = 0
    GA_BLK: int = 0        # filled by host_prep
    GB_BLK: int = 0

    def __post_init__(self):
        self.NBLK = -(-self.N // (self.NCORES * P))
        self.PC = self.NBLK * P
        self.NP = self.NCORES * self.PC
        self.F0 = self.HEADS * self.HID
        assert self.NP % 2 == 0
        assert self.NP // 2 < 32768, "half-table must fit int16 indices"


def chunk_plan(cfg: Cfg):
    out, b = [], 0
    while b < cfg.NBLK:
        out.append((b, min(cfg.CBLK, cfg.NBLK - b)))
        b += out[-1][1]
    return out


def host_prep(cfg: Cfg, x: np.ndarray, edge_index: np.ndarray, weights: dict):
    """Pure int / layout prep (no float arithmetic). Returns (in_maps, new2old)."""
    N, NC, NBLK, PC, NP = cfg.N, cfg.NCORES, cfg.NBLK, cfg.PC, cfg.NP
    src = edge_index[0].astype(np.int64)
    dst = edge_index[1].astype(np.int64)

    # balanced relabeling: snake-deal nodes by in-degree into NC*NBLK bins
    deg = np.bincount(dst, minlength=NP)
    order = np.argsort(-deg, kind="stable")
    nbins = NC * NBLK
    old2new = np.empty(NP, np.int64)
    for i0 in range(0, NP, nbins):
        row = order[i0:i0 + nbins]
        j = np.arange(len(row))
        if (i0 // nbins) % 2 == 1:
            j = nbins - 1 - j
        c, b = j % NC, j // NC
        old2new[row] = c * PC + b * P + (i0 // nbins)
    new2old = np.argsort(old2new)

    sn, dn = old2new[src], old2new[dst]
    HALF = NP // 2
    core_of = dn // PC
    blk_of = (dn % PC) // P
    half_of = (sn >= HALF).astype(np.int64)

    # bucket edges
    key = (core_of * NBLK + blk_of) * 2 + half_of
    eorder = np.argsort(key, kind="stable")
    bounds = np.searchsorted(key[eorder], np.arange(NC * NBLK * 2 + 1))

    def bucket(c, b, h):
        k = (c * NBLK + b) * 2 + h
        return eorder[bounds[k]:bounds[k + 1]]

    GA_BLK = GB_BLK = 1
    for c in range(NC):
        for b in range(NBLK):
            GA_BLK = max(GA_BLK, -(-len(bucket(c, b, 0)) // P))
            GB_BLK = max(GB_BLK, -(-len(bucket(c, b, 1)) // P))
    cfg.GA_BLK, cfg.GB_BLK = GA_BLK, GB_BLK

    chunks = chunk_plan(cfg)
    GT = NBLK * (GA_BLK + GB_BLK)
    L = GT * P

    # weights / consts: layout only
    w0 = np.asarray(weights["w0"], np.float32)
    w1 = np.asarray(weights["w1"], np.float32)
    A0 = np.zeros((cfg.F0, 2 * cfg.HEADS), np.float32)
    for h in range(cfg.HEADS):
        A0[h * cfg.HID:(h + 1) * cfg.HID, h] = weights["asrc0"][h]
        A0[h * cfg.HID:(h + 1) * cfg.HID, cfg.HEADS + h] = weights["adst0"][h]
    A1 = np.stack([np.asarray(weights["asrc1"][0]),
                   np.asarray(weights["adst1"][0])], axis=1).astype(np.float32)
    bn0 = np.concatenate([weights["bn0_g"], weights["bn0_b"], weights["bn0_m"],
                          weights["bn0_v"], weights["b0"]]).astype(
                              np.float32).reshape(1, -1)
    bn1 = np.concatenate([weights["bn1_g"], weights["bn1_b"], weights["bn1_m"],
                          weights["bn1_v"], weights["b1"]]).astype(
                              np.float32).reshape(1, -1)

    xp = np.zeros((NP, cfg.IN), np.float32)
    xp[old2new[:N]] = np.asarray(x, np.float32)
    xT = np.ascontiguousarray(xp.T)

    shared = {
        "xT": xT,
        "w0T": np.ascontiguousarray(w0.T),
        "w0": np.ascontiguousarray(w0),
        "a0": A0,
        "sk0T": np.ascontiguousarray(np.asarray(weights["skip0"], np.float32).T),
        "w1": np.ascontiguousarray(w1),
        "w1T": np.ascontiguousarray(w1.T),
        "a1": A1,
        "sk1T": np.ascontiguousarray(np.asarray(weights["skip1"], np.float32).T),
        "bn0": bn0,
        "bn1": bn1,
        "iotap": np.arange(P, dtype=np.float32).reshape(P, 1).astype(BF),
        "iotar": np.tile(np.arange(P, dtype=np.float32), (P, 1)).astype(BF),
        "ones1": np.ones((1, P), np.float32),
        "ident": np.eye(P, dtype=np.float32),
    }

    in_maps = []
    for c in range(NC):
        srcidx = np.zeros(L, np.int64)
        dloc = np.full(L, P, np.int64)
        pos = 0
        for (b0, nb) in chunks:
            for half in (0, 1):
                GBH = GA_BLK if half == 0 else GB_BLK
                for b in range(b0, b0 + nb):
                    sl = bucket(c, b, half)
                    k = len(sl)
                    srcidx[pos:pos + k] = sn[sl] - half * HALF
                    dloc[pos:pos + k] = dn[sl] % P
                    pos += GBH * P
        assert pos == L
        wr = srcidx.reshape(L // 16, 16).T.astype(np.int16)
        m = {
            "idx0": np.ascontiguousarray(np.tile(wr, (8, 1))),
            "dloc_pc": np.ascontiguousarray(
                dloc.reshape(GT, P).T.astype(np.float32)).astype(BF),
            "dloc_rep": np.ascontiguousarray(
                np.tile(dloc.astype(np.float32).astype(BF), (P, 1))),
            "xTloc": np.ascontiguousarray(xT[:, c * PC:(c + 1) * PC]),
        }
        m.update(shared)
        in_maps.append(m)
    return in_maps, new2old


def build_program(cfg: Cfg):
    nc = bacc.Bacc("TRN2", target_bir_lowering=False, num_swdge_queues=4)
    NBLK, PC, NP = cfg.NBLK, cfg.PC, cfg.NP
    F0, HID, HEADS = cfg.F0, cfg.HID, cfg.HEADS
    GA_BLK, GB_BLK = cfg.GA_BLK, cfg.GB_BLK
    GPB = GA_BLK + GB_BLK
    GT = NBLK * GPB
    L = GT * P
    HALF = NP // 2
    D0, D1 = 256, 128
    EPS = 1e-5
    CGMAX = cfg.CBLK * GPB

    t_in = {}
    for nm, shape, dt in [
        ("xT", [P, NP], F32), ("xTloc", [P, PC], F32),
        ("w0T", [P, F0], F32), ("w0", [F0, P], F32),
        ("a0", [F0, 2 * HEADS], F32), ("sk0T", [P, F0], F32),
        ("w1", [HID, F0], F32), ("w1T", [F0, HID], F32),
        ("a1", [HID, 2], F32), ("sk1T", [F0, HID], F32),
        ("bn0", [1, 5 * F0], F32), ("bn1", [1, 5 * HID], F32),
        ("iotap", [P, 1], BF16), ("iotar", [P, P], BF16),
        ("ones1", [1, P], F32), ("ident", [P, P], F32),
        ("idx0", [P, L // 16], I16),
        ("dloc_pc", [P, GT], BF16), ("dloc_rep", [P, GT * P], BF16),
    ]:
        t_in[nm] = nc.dram_tensor(nm, shape, dt, kind="ExternalInput")
    y = nc.dram_tensor("y", [PC, HID], F32, kind="ExternalOutput")
    table0 = nc.dram_tensor("table0", [NP, D0], BF16)
    ag_in = nc.dram_tensor("ag_in", [PC, D1], BF16)
    table1 = nc.dram_tensor("table1", [NP, D1], BF16, addr_space="Shared")

    chunks = chunk_plan(cfg)
    goff = np.cumsum([0] + [nb * GPB for (_, nb) in chunks]).tolist()

    with tile.TileContext(nc) as tc, ExitStack() as st:
        nc.gpsimd.load_library(library_config.mlp)
        cst = st.enter_context(tc.tile_pool(name="cst", bufs=1))
        sb = st.enter_context(tc.tile_pool(name="sb", bufs=2))
        sb1 = st.enter_context(tc.tile_pool(name="sb1", bufs=1))
        sb3 = st.enter_context(tc.tile_pool(name="sb3", bufs=3))
        psA = st.enter_context(tc.tile_pool(name="psA", bufs=2, space="PSUM"))
        psB = st.enter_context(tc.tile_pool(name="psB", bufs=2, space="PSUM"))
        psC = st.enter_context(tc.tile_pool(name="psC", bufs=2, space="PSUM"))

        def ld(nm):
            t = t_in[nm]
            s = cst.tile(list(t.shape), t.dtype, tag=f"c_{nm}")
            nc.sync.dma_start(out=s[:], in_=t[:])
            return s

        w0T_s, w0_s, a0_s, sk0T_s = ld("w0T"), ld("w0"), ld("a0"), ld("sk0T")
        w1_s, w1T_s, a1_s, sk1T_s = ld("w1"), ld("w1T"), ld("a1"), ld("sk1T")
        bn0_s, bn1_s = ld("bn0"), ld("bn1")
        iotap_s, iotar_s = ld("iotap"), ld("iotar")
        ones1_s, ident_s = ld("ones1"), ld("ident")

        # ccat0 [P, 2H] = w0^T(as lhsT) @ A0 ; ccat1 [P, 2]
        ccat0_p = psC.tile([P, 2 * HEADS], F32, tag="scr")
        nc.tensor.matmul(ccat0_p[:], lhsT=w0_s[:], rhs=a0_s[:], start=True, stop=True)
        ccat0_s = cst.tile([P, 2 * HEADS], F32, tag="ccat0")
        nc.vector.tensor_copy(out=ccat0_s[:], in_=ccat0_p[:])
        ccat1_p = psC.tile([P, 2], F32, tag="scr")
        nc.tensor.matmul(ccat1_p[:], lhsT=w1_s[:], rhs=a1_s[:], start=True, stop=True)
        ccat1_s = cst.tile([P, 2], F32, tag="ccat1")
        nc.vector.tensor_copy(out=ccat1_s[:], in_=ccat1_p[:])

        # BN: replicate flat param rows to 128 partitions, then compute
        # scale/shift on full tiles.
        def bn_prep(bn_s, W):
            reps = []
            for i in range(5):           # g, b, m, v, bias
                pt = psC.tile([P, W], F32, tag="scr")
                nc.tensor.matmul(pt[:], lhsT=ones1_s[:],
                                 rhs=bn_s[:, i * W:(i + 1) * W],
                                 start=True, stop=True)
                full = sb3.tile([P, W], F32, tag=f"bnrep{i}_{W}")
                nc.vector.tensor_copy(out=full[:], in_=pt[:])
                reps.append(full)
            g, b, m, v, bias = reps
            s_full = cst.tile([P, W], F32, tag=f"bns_{W}")
            nc.vector.tensor_scalar_add(out=s_full[:], in0=v[:], scalar1=EPS)
            nc.scalar.activation(s_full[:], s_full[:], AF.Sqrt)
            nc.vector.reciprocal(out=s_full[:], in_=s_full[:])
            nc.vector.tensor_mul(out=s_full[:], in0=s_full[:], in1=g[:])
            sh_full = cst.tile([P, W], F32, tag=f"bnsh_{W}")
            nc.vector.tensor_sub(out=sh_full[:], in0=bias[:], in1=m[:])
            nc.vector.tensor_mul(out=sh_full[:], in0=sh_full[:], in1=s_full[:])
            nc.vector.tensor_add(out=sh_full[:], in0=sh_full[:], in1=b[:])
            return s_full, sh_full

        s0_full, sh0_full = bn_prep(bn0_s, F0)
        s1_full, sh1_full = bn_prep(bn1_s, HID)

        # ---- phase 1: build table0 rows [h0 | asrc | zeros] (full, per core) ----
        stg0 = []
        for i in range(3):
            s = sb1.tile([P, D0], BF16, tag=f"stg0_{i}")
            nc.vector.memset(s[:, F0 + HEADS:D0], 0)
            stg0.append(s)
        for t in range(NP // P):
            xt = sb3.tile([P, P], F32, tag="xtile")
            nc.sync.dma_start(out=xt[:], in_=t_in["xT"][:, t * P:(t + 1) * P])
            h0p = psA.tile([P, F0], F32, tag="pout")
            nc.tensor.matmul(h0p[:], lhsT=xt[:], rhs=w0T_s[:], start=True, stop=True)
            a0p = psC.tile([P, 2 * HEADS], F32, tag="scr")
            nc.tensor.matmul(a0p[:], lhsT=xt[:], rhs=ccat0_s[:], start=True, stop=True)
            stage = stg0[t % 3]
            nc.scalar.copy(out=stage[:, 0:F0], in_=h0p[:])
            nc.vector.tensor_copy(out=stage[:, F0:F0 + HEADS], in_=a0p[:, 0:HEADS])
            nc.sync.dma_start(out=table0[t * P:(t + 1) * P, :], in_=stage[:])

        # ---- local adst0 per block ----
        adst0_b = cst.tile([P, NBLK * HEADS], BF16)
        for b in range(NBLK):
            xt = sb3.tile([P, P], F32, tag="xtile")
            nc.sync.dma_start(out=xt[:], in_=t_in["xTloc"][:, b * P:(b + 1) * P])
            a0p = psC.tile([P, 2 * HEADS], F32, tag="scr")
            nc.tensor.matmul(a0p[:], lhsT=xt[:], rhs=ccat0_s[:], start=True, stop=True)
            nc.vector.tensor_copy(out=adst0_b[:, b * HEADS:(b + 1) * HEADS],
                                  in_=a0p[:, HEADS:2 * HEADS])

        h1T = cst.tile([P, PC], F32)
        adst1_b = cst.tile([P, NBLK], BF16)
        skip1x = cst.tile([P, NBLK * HID], F32)

        def post_block(layer, b, pout):
            FC, H = (F0, HEADS) if layer == 0 else (HID, 1)
            R = FC // H
            den = sb3.tile([P, HEADS], F32, tag="den")
            nc.vector.tensor_scalar_add(out=den[:, 0:H], in0=pout[:, FC:FC + H],
                                        scalar1=1e-16)
            rec = sb3.tile([P, HEADS], F32, tag="rec")
            nc.vector.reciprocal(out=rec[:, 0:H], in_=den[:, 0:H])
            gat = sb3.tile([P, F0], F32, tag="gat")
            nc.vector.tensor_tensor(
                out=gat[:, 0:FC].rearrange("p (h r) -> p h r", r=R),
                in0=pout[:, 0:FC].rearrange("p (h r) -> p h r", r=R),
                in1=rec[:, 0:H].unsqueeze(2).to_broadcast([P, H, R]),
                op=ALU.mult)
            if layer == 0:
                s_full, sh_full = s0_full, sh0_full
                xt = sb3.tile([P, P], F32, tag="xtile")
                nc.sync.dma_start(out=xt[:], in_=t_in["xTloc"][:, b * P:(b + 1) * P])
                skp = psC.tile([P, F0], F32, tag="scr")
                nc.tensor.matmul(skp[:], lhsT=xt[:], rhs=sk0T_s[:],
                                 start=True, stop=True)
                skv = skp[:]
            else:
                s_full, sh_full = s1_full, sh1_full
                skv = skip1x[:, b * HID:(b + 1) * HID]
            nc.vector.tensor_mul(out=gat[:, 0:FC], in0=gat[:, 0:FC],
                                 in1=s_full[:, 0:FC])
            nc.vector.tensor_add(out=gat[:, 0:FC], in0=gat[:, 0:FC],
                                 in1=sh_full[:, 0:FC])
            nc.vector.tensor_add(out=gat[:, 0:FC], in0=gat[:, 0:FC], in1=skv)
            # elu = relu(x) + exp(min(x,0)) - 1
            mn = sb3.tile([P, F0], F32, tag="mn")
            nc.vector.tensor_scalar_min(out=mn[:, 0:FC], in0=gat[:, 0:FC],
                                        scalar1=0.0)
            nc.scalar.activation(mn[:, 0:FC], mn[:, 0:FC], AF.Exp)
            nc.vector.tensor_scalar_max(out=gat[:, 0:FC], in0=gat[:, 0:FC],
                                        scalar1=0.0)
            nc.vector.tensor_add(out=gat[:, 0:FC], in0=gat[:, 0:FC],
                                 in1=mn[:, 0:FC])
            nc.vector.tensor_scalar_add(out=gat[:, 0:FC], in0=gat[:, 0:FC],
                                        scalar1=-1.0)
            if layer == 0:
                tp = psC.tile([P, P], F32, tag="scr")
                nc.tensor.transpose(tp[:], gat[:, 0:F0], ident_s[:])
                nc.vector.tensor_copy(out=h1T[:, b * P:(b + 1) * P], in_=tp[:])
            else:
                yt = sb3.tile([P, HID], F32, tag="yt")
                nc.vector.tensor_copy(out=yt[:], in_=gat[:, 0:HID])
                nc.sync.dma_start(out=y[b * P:(b + 1) * P, :], in_=yt[:])

        def scatter_layer(layer):
            if layer == 0:
                table, D, FC, H, acol, adstb = table0, D0, F0, HEADS, F0, adst0_b
            else:
                table, D, FC, H, acol, adstb = table1, D1, HID, 1, HID, adst1_b
            R = FC // H
            for k, (b0, nb) in enumerate(chunks):
                CG = nb * GPB
                CGA = nb * GA_BLK
                gof = goff[k]
                sof = gof * P
                gath_full = sb.tile([P, CGMAX * D0], BF16, tag="gath")
                gath = gath_full[:, 0:CGMAX * D].rearrange("p (g d) -> p g d", d=D)
                idx_t = sb.tile([P, CGMAX * 8], I16, tag="idxt")
                nc.sync.dma_start(out=idx_t[:, 0:CG * 8],
                                  in_=t_in["idx0"][:, sof // 16:(sof + CG * P) // 16])
                nc.gpsimd.dma_gather(
                    out_ap=gath[:, 0:CGA, :], in_ap=table[0:HALF, :],
                    idxs_ap=idx_t[:, 0:CGA * 8], num_idxs=CGA * P,
                    num_idxs_reg=CGA * P, elem_size=D, single_packet=False,
                    queue_num=(2 * k) % 4)
                nc.gpsimd.dma_gather(
                    out_ap=gath[:, CGA:CG, :], in_ap=table[HALF:NP, :],
                    idxs_ap=idx_t[:, CGA * 8:CG * 8], num_idxs=(CG - CGA) * P,
                    num_idxs_reg=(CG - CGA) * P, elem_size=D, single_packet=False,
                    queue_num=(2 * k + 1) % 4)
                dpc = sb.tile([P, CGMAX], BF16, tag="dpc")
                nc.sync.dma_start(out=dpc[:, 0:CG],
                                  in_=t_in["dloc_pc"][:, gof:gof + CG])
                drep = sb1.tile([P, CGMAX * P], BF16, tag="drep")
                nc.sync.dma_start(out=drep[:, 0:CG * P],
                                  in_=t_in["dloc_rep"][:, gof * P:(gof + CG) * P])
                m01 = sb.tile([P, CGMAX * P], BF16, tag="m01")
                nc.vector.tensor_tensor(
                    out=m01[:, 0:CG * P].rearrange("p (g i) -> p g i", i=P),
                    in0=dpc[:, 0:CG].unsqueeze(2).to_broadcast([P, CG, P]),
                    in1=iotar_s[:].unsqueeze(1).to_broadcast([P, CG, P]),
                    op=ALU.is_equal)
                m01t = sb.tile([P, CGMAX * P], BF16, tag="m01t")
                nc.vector.tensor_tensor(
                    out=m01t[:, 0:CG * P],
                    in0=iotap_s[:].to_broadcast([P, CG * P]),
                    in1=drep[:, 0:CG * P],
                    op=ALU.is_equal)
                pex = psB.tile([P, CGMAX * HEADS], F32, tag="pex")
                for g in range(CG):
                    b = (b0 + g // GA_BLK) if g < CGA else (b0 + (g - CGA) // GB_BLK)
                    nc.tensor.matmul(
                        pex[:, g * H:(g + 1) * H],
                        lhsT=m01t[:, g * P:(g + 1) * P],
                        rhs=adstb[:, b * H:(b + 1) * H],
                        start=True, stop=True)
                asrcf = sb.tile([P, CGMAX * HEADS], F32, tag="asrcf")
                nc.scalar.copy(out=asrcf[:, 0:CG * H].rearrange(
                                   "p (g h) -> p g h", h=H),
                               in_=gath[:, 0:CG, acol:acol + H])
                alpha = sb.tile([P, CGMAX * HEADS], F32, tag="alpha")
                nc.vector.tensor_add(out=alpha[:, 0:CG * H], in0=pex[:, 0:CG * H],
                                     in1=asrcf[:, 0:CG * H])
                alpha2 = sb.tile([P, CGMAX * HEADS], F32, tag="alpha2")
                nc.vector.tensor_scalar_mul(out=alpha2[:, 0:CG * H],
                                            in0=alpha[:, 0:CG * H], scalar1=0.2)
                nc.vector.tensor_max(out=alpha[:, 0:CG * H], in0=alpha[:, 0:CG * H],
                                     in1=alpha2[:, 0:CG * H])
                exb = sb.tile([P, CGMAX * HEADS], BF16, tag="exb")
                nc.scalar.activation(exb[:, 0:CG * H], alpha[:, 0:CG * H], AF.Exp)
                for bb in range(nb):
                    b = b0 + bb
                    pout = psA.tile([P, F0 + HEADS], F32, tag="pout")
                    glist = ([bb * GA_BLK + j for j in range(GA_BLK)] +
                             [CGA + bb * GB_BLK + j for j in range(GB_BLK)])
                    for gi, g in enumerate(glist):
                        msg = sb3.tile([P, F0 + HEADS], BF16, tag="msg")
                        nc.vector.tensor_tensor(
                            out=msg[:, 0:FC].rearrange("p (h r) -> p h r", r=R),
                            in0=gath[:, g, 0:FC].rearrange("p (h r) -> p h r", r=R),
                            in1=exb[:, g * H:(g + 1) * H].unsqueeze(2)
                                .to_broadcast([P, H, R]),
                            op=ALU.mult)
                        nc.scalar.copy(out=msg[:, FC:FC + H],
                                       in_=exb[:, g * H:(g + 1) * H])
                        first = gi == 0
                        last = gi == len(glist) - 1
                        nc.tensor.matmul(pout[:, 0:FC + H],
                                         lhsT=m01[:, g * P:(g + 1) * P],
                                         rhs=msg[:, 0:FC + H],
                                         start=first, stop=last)
                    post_block(layer, b, pout)

        scatter_layer(0)

        # ---- phase 3: layer-1 per-node quantities + AllGather ----
        wc1 = cst.tile([F0, HID + 2], F32)
        nc.vector.tensor_copy(out=wc1[:, 0:HID], in_=w1T_s[:])
        nc.vector.tensor_copy(out=wc1[:, HID:HID + 2], in_=ccat1_s[:])
        stg1 = []
        for i in range(2):
            s = sb1.tile([P, D1], BF16, tag=f"stg1_{i}")
            nc.vector.memset(s[:, HID + 2:D1], 0)
            stg1.append(s)
        for b in range(NBLK):
            t1p = psC.tile([P, HID + 2], F32, tag="scr")
            nc.tensor.matmul(t1p[:], lhsT=h1T[:, b * P:(b + 1) * P], rhs=wc1[:],
                             start=True, stop=True)
            stage = stg1[b % 2]
            nc.vector.tensor_copy(out=stage[:, 0:HID + 2], in_=t1p[:])
            nc.sync.dma_start(out=ag_in[b * P:(b + 1) * P, :], in_=stage[:])
            nc.vector.tensor_copy(out=adst1_b[:, b:b + 1],
                                  in_=t1p[:, HID + 1:HID + 2])
            skp = psC.tile([P, HID], F32, tag="scr")
            nc.tensor.matmul(skp[:], lhsT=h1T[:, b * P:(b + 1) * P], rhs=sk1T_s[:],
                             start=True, stop=True)
            nc.vector.tensor_copy(out=skip1x[:, b * HID:(b + 1) * HID], in_=skp[:])

        nc.gpsimd.collective_compute(
            "AllGather", ALU.bypass,
            replica_groups=[list(range(cfg.NCORES))],
            ins=[ag_in[:]], outs=[table1[:]])

        scatter_layer(1)

    nc.compile()
    return nc




# ======================================================================
# Self-contained kernel entry point.
# kernel(**inputs) takes FULL unsharded inputs (as from setup_inputs())
# and returns the FULL [50000, 16] float32 output. Internally shards
# across 8 NeuronCores (dst-node sharding), runs the Bass program via
# bass_utils.run_bass_kernel_spmd, and reassembles the output.
# ======================================================================
from concourse import bass_utils as _bass_utils

_CACHE = {}


def kernel(**inputs):
    x = np.asarray(inputs["x"], np.float32)
    edge_index = np.asarray(inputs["edge_index"])
    cfg = Cfg(N=50000, E=int(edge_index.shape[1]))
    in_maps, new2old = host_prep(cfg, x, edge_index, inputs)
    key = "prog"
    # topology-dependent program: cache on (GA_BLK, GB_BLK) and edge hash
    sig = (cfg.GA_BLK, cfg.GB_BLK)
    if _CACHE.get("sig") != sig:
        _CACHE["nc"] = build_program(cfg)
        _CACHE["sig"] = sig
    nc = _CACHE["nc"]
    res = _bass_utils.run_bass_kernel_spmd(
        nc, in_maps, core_ids=list(range(cfg.NCORES)))
    yfull = np.concatenate([res.results[c]["y"] for c in range(cfg.NCORES)],
                           axis=0)
    out = np.zeros((cfg.N, cfg.HID), np.float32)
    valid = new2old < cfg.N
    out[new2old[valid]] = yfull[valid]
    return out



# revision 3
# speedup vs baseline: 1.3286x; 1.3286x over previous
"""GAT 2-layer kernel v2 for trn2 — per-partition CSR edge layout.

- Nodes relabeled by in-degree rank; blocks of 128 nodes have near-uniform
  degree. Block g -> core g%8, local block g//8. Slot (p, j) of block lb
  holds the j-th in-edge of dst node p: the segment-sum "scatter" becomes an
  identity-matmul PSUM accumulation and a_dst a free broadcast. No one-hot
  matrices.
- Tables pair-packed (2 nodes/row) so gather indices fit int16:
  table0 rows 768B ([h0|asrc] x2), table1 rows 256B ([t1|a1] x2 + pad).
  Pair halves selected by folding the select bit into edge weights.
- BN folded into weights/shifts host-side; layer-0 ELU's "-1" folded into
  layer-1 constants.
- Layer-1 node table distributed by 3 chunked AllGathers overlapped with the
  layer-0 tail.
"""
import numpy as np
import ml_dtypes
from dataclasses import dataclass, field
from contextlib import ExitStack

from concourse import bass, mybir, tile, bacc
from concourse import library_config

P = 128
F32 = mybir.dt.float32
BF16 = mybir.dt.bfloat16
I16 = mybir.dt.int16
AF = mybir.ActivationFunctionType
ALU = mybir.AluOpType
BF = ml_dtypes.bfloat16
NEG = -1e4


@dataclass
class Cfg:
    N: int = 50000
    E: int = 800000
    IN: int = 128
    HID: int = 16
    HEADS: int = 8
    NCORES: int = 8
    CS: int = 36                  # max slot-columns per gather chunk
    NBLK: int = 0
    PC: int = 0
    NP: int = 0
    F0: int = 0
    NPP: int = 0
    J: list = field(default_factory=list)
    cum: list = field(default_factory=list)
    chunks: list = field(default_factory=list)
    LB: tuple = (0, 16, 32, 49)   # allgather chunk bounds (multiples of 4)
    a1bias: float = 0.0

    def __post_init__(self):
        self.NBLK = -(-self.N // (self.NCORES * P))
        self.PC = self.NBLK * P
        self.NP = self.NCORES * self.PC
        self.F0 = self.HEADS * self.HID
        self.NPP = self.NP // 2
        assert self.NPP + 1 < 32768


def _wrap_idx(idx_lin):
    wr = idx_lin.reshape(-1, 16).T.astype(np.int16)
    return np.ascontiguousarray(np.tile(wr, (8, 1)))


def host_prep(cfg: Cfg, x, edge_index, weights):
    N, NC, NBLK, PC, NP = cfg.N, cfg.NCORES, cfg.NBLK, cfg.PC, cfg.NP
    F0, HID, HEADS = cfg.F0, cfg.HID, cfg.HEADS
    src = edge_index[0].astype(np.int64)
    dst = edge_index[1].astype(np.int64)

    deg = np.bincount(dst, minlength=NP)
    order = np.argsort(-deg, kind="stable")
    rank = np.empty(NP, np.int64)
    rank[order] = np.arange(NP)
    gblk = rank // P
    pos = rank % P
    core = gblk % NC
    lblk = gblk // NC
    old2new = core * PC + lblk * P + pos
    new2old = np.argsort(old2new)

    bmax = deg[order].reshape(-1, P).max(1)
    J = np.maximum(bmax.reshape(NBLK, NC).max(1).astype(int), 1)
    cfg.CS = max(cfg.CS, int(J.max()))
    cfg.J = [int(v) for v in J]
    cum = np.concatenate([[0], np.cumsum(J)]).astype(int)
    cfg.cum = [int(v) for v in cum]
    S = int(cum[-1])

    chunks = []
    lb0 = 0
    while lb0 < NBLK:
        lb1 = lb0 + 1
        while lb1 < NBLK and cum[lb1 + 1] - cum[lb0] <= cfg.CS:
            lb1 += 1
        chunks.append((lb0, lb1))
        lb0 = lb1
    cfg.chunks = chunks

    sn, dn = rank[src], rank[dst]
    d_g, d_p = dn // P, dn % P
    d_c, d_lb = d_g % NC, d_g // NC
    eorder = np.argsort(dn, kind="stable")
    dns = dn[eorder]
    starts = np.searchsorted(dns, np.arange(NP))
    jidx = np.empty(cfg.E, np.int64)
    jidx[eorder] = np.arange(cfg.E) - starts[dns]
    s_col = cum[d_lb] + jidx

    s_g, s_p = sn // P, sn % P
    s_c, s_lb = s_g % NC, s_g // NC
    bsel1 = (s_p % 2).astype(np.int64)      # node parity (layer-1 pairs)

    LB = np.array(cfg.LB)
    kk = np.searchsorted(LB, s_lb, side="right") - 1
    ag_base = np.concatenate([[0], np.cumsum(NC * (LB[1:] - LB[:-1]) * P)])
    wid = LB[kk + 1] - LB[kk]
    pos1 = ag_base[kk] + (s_c * wid + (s_lb - LB[kk])) * P + s_p
    idx1_val = pos1 // 2
    assert np.all((pos1 % 2) == bsel1)

    # --- weight folds (host float on weights only) ---
    w0 = np.asarray(weights["w0"], np.float32)
    asrc0 = np.asarray(weights["asrc0"], np.float32)
    adst0 = np.asarray(weights["adst0"], np.float32)
    s0 = np.asarray(weights["bn0_g"], np.float32) / np.sqrt(
        np.asarray(weights["bn0_v"], np.float32) + 1e-5)
    w0p = w0 * s0[:, None]
    acat0 = np.zeros((cfg.IN, 2 * HEADS), np.float32)
    for h in range(HEADS):
        acat0[:, h] = asrc0[h] @ w0[h * HID:(h + 1) * HID, :]
        acat0[:, HEADS + h] = adst0[h] @ w0[h * HID:(h + 1) * HID, :]
    # c-major feature order: new index c*HEADS+h <- old h*HID+c
    perm = (np.arange(F0).reshape(HEADS, HID).T.reshape(-1))
    wcat0 = np.concatenate([w0p.T[:, perm], acat0], axis=1)
    shift0 = (s0 * np.asarray(weights["b0"], np.float32)
              + np.asarray(weights["bn0_b"], np.float32)
              - s0 * np.asarray(weights["bn0_m"], np.float32))[perm]
    sk0T = np.ascontiguousarray(
        np.asarray(weights["skip0"], np.float32).T[:, perm])

    w1 = np.asarray(weights["w1"], np.float32)
    s1 = np.asarray(weights["bn1_g"], np.float32) / np.sqrt(
        np.asarray(weights["bn1_v"], np.float32) + 1e-5)
    w1p = w1 * s1[:, None]
    ccat1 = np.stack([np.asarray(weights["asrc1"], np.float32)[0] @ w1,
                      np.asarray(weights["adst1"], np.float32)[0] @ w1],
                     axis=1)
    sk1 = np.asarray(weights["skip1"], np.float32)
    # layer-1 input features arrive c-major: permute rows
    wcat1 = np.concatenate([w1p.T, ccat1, sk1.T], axis=1)[perm]
    shift1 = (s1 * np.asarray(weights["b1"], np.float32)
              + np.asarray(weights["bn1_b"], np.float32)
              - s1 * np.asarray(weights["bn1_m"], np.float32)
              - w1p.sum(1) - sk1.sum(1))
    shift1pad = np.zeros((1, 2 * HID + 2), np.float32)
    shift1pad[0, HID + 2:] = shift1
    cfg.a1bias = float(-(ccat1[:, 0].sum() + ccat1[:, 1].sum()))

    xT = np.zeros((cfg.IN, NP), np.float32)
    xT[:, old2new[:N]] = np.asarray(x, np.float32).T
    xTb = xT.astype(BF)

    shared = {
        "wcat0": wcat0.astype(BF),
        "sk0Tb": sk0T.astype(BF),
        "shift0r": shift0.reshape(1, -1).astype(np.float32),
        "wcat1": wcat1.astype(BF),
        "shift1pad": shift1pad,
        "ones1": np.ones((1, P), np.float32),
        "identb": np.eye(P, dtype=np.float32).astype(BF),
    }

    in_maps = []
    for c in range(NC):
        m_e = d_c == c
        Bsrc = np.where(s_c == c, s_lb,
                        NBLK * (1 + ((s_c - c - 1) % NC)) + s_lb)
        # table0 rows: partition-major block pairs: row = p*(NB/2) + B//2
        NBH = (NBLK * NC) // 2
        idx0_val = s_p * NBH + Bsrc // 2
        bsel0 = Bsrc % 2                     # block parity (layer-0 pairs)

        idx0_lin = np.full(S * P, cfg.NPP, np.int64)
        idx1_lin = np.full(S * P, cfg.NPP, np.int64)
        bs2 = np.zeros((P, S, 2), np.float32)
        bs2b = np.zeros((P, S, 2), np.float32)
        sc, sp = s_col[m_e], d_p[m_e]
        lin = sc * P + sp
        idx0_lin[lin] = idx0_val[m_e]
        idx1_lin[lin] = idx1_val[m_e]
        bs2[sp, sc, 0] = 1.0 - bsel0[m_e]
        bs2[sp, sc, 1] = bsel0[m_e]
        bs2b[sp, sc, 0] = 1.0 - bsel1[m_e]
        bs2b[sp, sc, 1] = bsel1[m_e]

        colord = np.empty(NP, np.int64)
        for k in range(NC):
            cs = (c + k) % NC
            colord[k * PC:(k + 1) * PC] = cs * PC + np.arange(PC)
        m = {
            "xTc": np.ascontiguousarray(xTb[:, colord]),
            "idx0s": _wrap_idx(idx0_lin),
            "idx1s": _wrap_idx(idx1_lin),
            "bsel2": np.ascontiguousarray(bs2.reshape(P, S * 2).astype(BF)),
            "bsel2b": np.ascontiguousarray(bs2b.reshape(P, S * 2).astype(BF)),
        }
        m.update(shared)
        in_maps.append(m)
    return in_maps, new2old


def _apd(base, dims):
    """AP with base's tensor/offset/partition-dim and explicit free dims."""
    return bass.AP(base.tensor, base.offset,
                   [list(base.ap[0])] + [list(d) for d in dims])


def build_program(cfg: Cfg, force_queue0: bool = False, dbg: bool = False):
    nc = bacc.Bacc("TRN2", target_bir_lowering=False, num_swdge_queues=4)
    qn = (lambda q: 0) if force_queue0 else (lambda q: q % 4)
    NBLK, PC, NP, NPP = cfg.NBLK, cfg.PC, cfg.NP, cfg.NPP
    F0, HID, HEADS, NC = cfg.F0, cfg.HID, cfg.HEADS, cfg.NCORES
    J, cum, chunks, LB = cfg.J, cfg.cum, cfg.chunks, list(cfg.LB)
    S = cum[-1]
    CS = cfg.CS
    JMAX = max(J)
    NB = NP // P
    ROW0 = 384            # table0 pair row elems (768B)
    ROW1 = 128            # table1 pair row elems (256B)
    AGW = HID + 1         # t1|a1 payload per node
    FH = F0 + HEADS

    t_in = {}
    for nm, shape, dt in [
        ("xTc", [P, NP], BF16),
        ("idx0s", [P, S * 8], I16), ("idx1s", [P, S * 8], I16),
        ("bsel2", [P, S * 2], BF16), ("bsel2b", [P, S * 2], BF16),
        ("wcat0", [P, F0 + 2 * HEADS], BF16),
        ("sk0Tb", [P, P], BF16),
        ("shift0r", [1, P], F32),
        ("wcat1", [P, 2 * HID + 2], BF16),
        ("shift1pad", [1, 2 * HID + 2], F32),
        ("ones1", [1, P], F32),
        ("identb", [P, P], BF16),
    ]:
        t_in[nm] = nc.dram_tensor(nm, shape, dt, kind="ExternalInput")
    y = nc.dram_tensor("y", [PC, HID], F32, kind="ExternalOutput")
    dk = dict(kind="ExternalOutput") if dbg else {}
    table0 = nc.dram_tensor("table0", [(NPP + 1) * ROW0], BF16, **dk)
    ag_in = nc.dram_tensor("ag_in", [PC, AGW], BF16)
    t1c = nc.dram_tensor("t1c", [NP, AGW], BF16, addr_space="Shared")
    table1 = nc.dram_tensor("table1", [(NPP + 1) * ROW1], BF16, **dk)

    t0v = table0[:].rearrange("(r s) -> r s", s=ROW0)
    t1v = table1[:].rearrange("(r s) -> r s", s=ROW1)
    szs = [NC * (LB[k + 1] - LB[k]) * P for k in range(len(LB) - 1)]
    ag_base = np.concatenate([[0], np.cumsum(szs)]).astype(int).tolist()

    gsems = [nc.alloc_semaphore(f"gsem{q}") for q in range(4)]
    with tile.TileContext(nc) as tc, ExitStack() as st:
        nc.gpsimd.load_library(library_config.mlp)
        cst = st.enter_context(tc.tile_pool(name="cst", bufs=1))
        xtp = st.enter_context(tc.tile_pool(name="xtp", bufs=2))
        stp = st.enter_context(tc.tile_pool(name="stp", bufs=2))
        gp = st.enter_context(tc.tile_pool(name="gp", bufs=2))
        mp = st.enter_context(tc.tile_pool(name="mp", bufs=2))
        wp = st.enter_context(tc.tile_pool(name="wp", bufs=2))
        psA = st.enter_context(tc.tile_pool(name="psA", bufs=2, space="PSUM"))
        psB = st.enter_context(tc.tile_pool(name="psB", bufs=2, space="PSUM"))
        psC = st.enter_context(tc.tile_pool(name="psC", bufs=2, space="PSUM"))

        def ld(nm):
            t = t_in[nm]
            s = cst.tile(list(t.shape), t.dtype, tag=f"c_{nm}")
            nc.sync.dma_start(out=s[:], in_=t[:])
            return s

        wcat0_s, sk0T_s, wcat1_s = ld("wcat0"), ld("sk0Tb"), ld("wcat1")
        shift0r_s, shift1p_s = ld("shift0r"), ld("shift1pad")
        ones1_s, ident_s = ld("ones1"), ld("identb")
        idx0_s, bsel2_s, bsel2b_s = ld("idx0s"), ld("bsel2"), ld("bsel2b")
        idx1_s = idx0_s

        xloc = cst.tile([P, PC], BF16)
        nc.sync.dma_start(out=xloc[:], in_=t_in["xTc"][:, 0:PC])
        h1T = cst.tile([P, PC], BF16)
        adst0_b = cst.tile([P, NBLK * HEADS], F32)
        adst1_b = cst.tile([P, NBLK], F32)
        skip1x = cst.tile([P, NBLK * HID], F32)
        a1bias_t = cst.tile([P, 1], F32)
        nc.vector.memset(a1bias_t[:], cfg.a1bias)

        # pad pair rows
        padt = cst.tile([P, ROW0], BF16)
        nc.vector.memset(padt[:], 0)
        nc.vector.memset(padt[:, F0:FH], NEG)
        nc.vector.memset(padt[:, FH + F0:2 * FH], NEG)
        nc.sync.dma_start(out=t0v[NPP:NPP + 1, :], in_=padt[0:1, :])
        pad1 = cst.tile([P, ROW1], BF16)
        nc.vector.memset(pad1[:], 0)
        nc.vector.memset(pad1[:, HID:HID + 1], NEG)
        nc.vector.memset(pad1[:, AGW + HID:AGW + HID + 1], NEG)
        nc.sync.dma_start(out=t1v[NPP:NPP + 1, :], in_=pad1[0:1, :])

        # ---- P0: build table0 (all NB blocks) ----
        BG = 16
        for bg in range(0, NB, BG):
            nb = min(BG, NB - bg)
            xt = xtp.tile([P, BG * P], BF16, tag="xt")
            nc.sync.dma_start(out=xt[:, 0:nb * P],
                              in_=t_in["xTc"][:, bg * P:(bg + nb) * P])
            stage = stp.tile([P, BG // 2, ROW0], BF16, tag="stage")
            for k in range(nb):
                B = bg + k
                pt = psA.tile([P, 2 * FH], F32, tag="pout")
                nc.tensor.matmul(pt[:, 0:F0 + 2 * HEADS],
                                 lhsT=xt[:, k * P:(k + 1) * P],
                                 rhs=wcat0_s[:], start=True, stop=True)
                if k % 2 == 0:
                    nc.vector.tensor_copy(
                        out=stage[:, k // 2, (k % 2) * FH:(k % 2 + 1) * FH],
                        in_=pt[:, 0:FH])
                else:
                    nc.scalar.copy(
                        out=stage[:, k // 2, (k % 2) * FH:(k % 2 + 1) * FH],
                        in_=pt[:, 0:FH])
                if B < NBLK:
                    nc.vector.tensor_copy(
                        out=adst0_b[:, B * HEADS:(B + 1) * HEADS],
                        in_=pt[:, FH:F0 + 2 * HEADS])
            # one partition-major store: partition p owns rows
            # [p*NBH + bg/2, +nb/2) -- contiguous 768B*nb/2 run
            NBH = NB // 2
            t0pm = table0[0:P * NBH * ROW0].rearrange(
                "(p r) -> p r", r=NBH * ROW0)
            nc.sync.dma_start(
                out=t0pm[:, (bg // 2) * ROW0:(bg // 2 + nb // 2) * ROW0],
                in_=stage[:, 0:nb // 2, :].rearrange("p q s -> p (q s)"))

        # ---- P1: layer-0 scatter + fused layer-1 prep + chunked AG ----
        GBK = 8
        state = {"agk": 0, "ags": None, "ys": None, "fs": None, "skpc": None,
                 "red4": None}
        t1x = cst.tile([P, NBLK, HID + 1], F32)      # [adst1 | skip1+shift]
        adst1_x = cst.tile([P, S], BF16)

        def l1_prep(lb):
            t1p = psC.tile([P, 2 * HID + 2], F32, tag="t1p")
            nc.tensor.matmul(t1p[:], lhsT=ones1_s[:], rhs=shift1p_s[:],
                             start=True, stop=False)
            nc.tensor.matmul(t1p[:], lhsT=h1T[:, lb * P:(lb + 1) * P],
                             rhs=wcat1_s[:], start=False, stop=True)
            if lb % 4 == 0:
                state["ags"] = stp.tile([P, 4, AGW], BF16, tag="ags",
                                        name="ags")
            ags = state["ags"]
            nc.scalar.copy(out=ags[:, lb % 4, :], in_=t1p[:, 0:AGW])
            nc.vector.tensor_copy(out=t1x[:, lb, :],
                                  in_=t1p[:, HID + 1:2 * HID + 2])
            nc.scalar.copy(
                out=adst1_x[:, cum[lb]:cum[lb + 1]],
                in_=t1p[:, HID + 1:HID + 2].to_broadcast([P, J[lb]]))
            if lb % 4 == 3 or lb == NBLK - 1:
                b0 = lb - lb % 4
                nc.sync.dma_start(
                    out=ag_in[b0 * P:(lb + 1) * P, :].rearrange(
                        "(b p) f -> p b f", p=P),
                    in_=ags[:, 0:lb % 4 + 1, :])
            agk = state["agk"]
            if agk < len(LB) - 1 and lb == LB[agk + 1] - 1:
                r0, r1 = LB[agk] * P, LB[agk + 1] * P
                nc.gpsimd.collective_compute(
                    "AllGather", ALU.bypass,
                    replica_groups=[list(range(NC))],
                    ins=[ag_in[r0:r1, :]],
                    outs=[t1c[ag_base[agk]:ag_base[agk + 1], :]])
                npr = (ag_base[agk + 1] - ag_base[agk]) // 2
                pr0 = ag_base[agk] // 2
                nc.sync.dma_start(
                    out=t1v[pr0:pr0 + npr, 0:2 * AGW],
                    in_=t1c[ag_base[agk]:ag_base[agk + 1], :].rearrange(
                        "(r two) f -> r (two f)", two=2))
                state["agk"] = agk + 1

        def tail0(g0, gn):
            fs, skpc = state["fs"], state["skpc"]
            # den += eps ; rec = 1/den  (dims (b, h))
            nc.vector.tensor_scalar_add(
                out=_apd(fs[:, 0:1, F0:F0 + 1], [[FH, gn], [1, HEADS]]),
                in0=_apd(fs[:, 0:1, F0:F0 + 1], [[FH, gn], [1, HEADS]]),
                scalar1=1e-16)
            recc = wp.tile([P, GBK, HEADS], F32, tag="recc")
            nc.vector.reciprocal(
                out=recc[:, 0:gn, :],
                in_=_apd(fs[:, 0:1, F0:F0 + 1], [[FH, gn], [1, HEADS]]))
            # v = num * rec  (b, c, h) in place
            nc.vector.tensor_tensor(
                out=_apd(fs[:, 0:1, 0:1],
                         [[FH, gn], [HEADS, HID], [1, HEADS]]),
                in0=_apd(fs[:, 0:1, 0:1],
                         [[FH, gn], [HEADS, HID], [1, HEADS]]),
                in1=_apd(recc[:, 0:1, 0:1],
                         [[HEADS, gn], [0, HID], [1, HEADS]]),
                op=ALU.mult)
            # v += skip&shift
            nc.vector.tensor_tensor(
                out=_apd(fs[:, 0:1, 0:1], [[FH, gn], [1, F0]]),
                in0=_apd(fs[:, 0:1, 0:1], [[FH, gn], [1, F0]]),
                in1=skpc[:, 0:gn, :], op=ALU.add)
            # elu' = relu(v) + exp(-relu(-v))
            r2c = wp.tile([P, GBK, F0], BF16, tag="r2c")
            nc.scalar.activation(
                r2c[:, 0:gn, :],
                _apd(fs[:, 0:1, 0:1], [[FH, gn], [1, F0]]), AF.Relu)
            r1c = wp.tile([P, GBK, F0], BF16, tag="r1c")
            nc.scalar.activation(
                r1c[:, 0:gn, :],
                _apd(fs[:, 0:1, 0:1], [[FH, gn], [1, F0]]),
                AF.Relu, scale=-1.0)
            nc.scalar.activation(r1c[:, 0:gn, :], r1c[:, 0:gn, :],
                                 AF.Exp, scale=-1.0)
            nc.vector.tensor_add(out=r2c[:, 0:gn, :], in0=r2c[:, 0:gn, :],
                                 in1=r1c[:, 0:gn, :])
            for bi in range(gn):
                lb = g0 + bi
                tp = psC.tile([P, P], BF16, tag="tp")
                nc.tensor.transpose(tp[:], r2c[:, bi, :], ident_s[:])
                nc.scalar.copy(out=h1T[:, lb * P:(lb + 1) * P], in_=tp[:])
                l1_prep(lb)

        for ci, (lb0, lb1) in enumerate(chunks):
            scol, ncol = cum[lb0], cum[lb1] - cum[lb0]
            gt = gp.tile([P, CS, ROW0], BF16, tag="gath")
            nc.gpsimd.dma_gather(
                out_ap=gt[:, 0:ncol, :], in_ap=t0v[:, :],
                idxs_ap=idx0_s[:, scol * 8:(scol + ncol) * 8],
                num_idxs=ncol * P, num_idxs_reg=ncol * P,
                elem_size=ROW0, single_packet=False, queue_num=qn(ci))
            alc = wp.tile([P, CS, 2, HEADS], BF16, tag="alc")
            for lb in range(lb0, lb1):
                Jb, off = J[lb], cum[lb] - scol
                adb = adst0_b[:, lb * HEADS:(lb + 1) * HEADS]
                for t in range(2):
                    nc.vector.tensor_tensor(
                        out=alc[:, off:off + Jb, t, :],
                        in0=gt[:, off:off + Jb, t * FH + F0:(t + 1) * FH],
                        in1=adb.unsqueeze(1).to_broadcast([P, Jb, HEADS]),
                        op=ALU.add)
            avf = alc[:, 0:ncol, :, :].rearrange("p s t h -> p (s t h)")
            nc.scalar.activation(avf, avf, AF.Prelu, alpha=0.2)
            nc.scalar.activation(avf, avf, AF.Exp)
            ws = wp.tile([P, CS, 2, HEADS], BF16, tag="ws")
            nc.vector.tensor_tensor(
                out=ws[:, 0:ncol, :, :], in0=alc[:, 0:ncol, :, :],
                in1=bsel2_s[:, scol * 2:(scol + ncol) * 2]
                    .rearrange("p (s t) -> p s t", t=2)
                    .unsqueeze(3).to_broadcast([P, ncol, 2, HEADS]),
                op=ALU.mult)
            msg = mp.tile([P, CS, 2 * FH], BF16, tag="msg")
            for t in range(2):
                nc.vector.tensor_tensor(
                    out=_apd(msg[:, 0:1, t * FH:t * FH + 1],
                             [[2 * FH, ncol], [HEADS, HID], [1, HEADS]]),
                    in0=_apd(gt[:, 0:1, t * FH:t * FH + 1],
                             [[ROW0, ncol], [HEADS, HID], [1, HEADS]]),
                    in1=_apd(ws[:, 0:1, t, 0:1],
                             [[2 * HEADS, ncol], [0, HID], [1, HEADS]]),
                    op=ALU.mult)
            nc.scalar.copy(
                out=_apd(msg[:, 0:1, F0:F0 + 1],
                         [[2 * FH, ncol], [FH, 2], [1, HEADS]]),
                in_=ws[:, 0:ncol, :, :])
            for lb in range(lb0, lb1):
                Jb, off = J[lb], cum[lb] - scol
                if lb % GBK == 0:
                    state["fs"] = wp.tile([P, GBK, FH], F32, tag="fs",
                                          name="fs")
                    state["skpc"] = wp.tile([P, GBK, F0], F32, tag="skpc",
                                            name="skpc")
                gi = lb % GBK
                pout = psA.tile([P, 2 * FH], F32, tag="pout")
                for j in range(Jb):
                    nc.tensor.matmul(pout[:], lhsT=ident_s[:],
                                     rhs=msg[:, off + j, :],
                                     start=(j == 0), stop=(j == Jb - 1))
                # fold pair halves -> fs[:, gi, 0:FH]
                nc.vector.tensor_reduce(
                    out=state["fs"][:, gi, 0:FH],
                    in_=_apd(pout[:, 0:1], [[1, FH], [FH, 2]]),
                    axis=mybir.AxisListType.X, op=ALU.add)
                skp = psB.tile([P, P], F32, tag="skp")
                nc.tensor.matmul(skp[:], lhsT=ones1_s[:], rhs=shift0r_s[:],
                                 start=True, stop=False)
                nc.tensor.matmul(skp[:], lhsT=xloc[:, lb * P:(lb + 1) * P],
                                 rhs=sk0T_s[:], start=False, stop=True)
                nc.scalar.copy(out=state["skpc"][:, gi, :], in_=skp[:])
                if lb % GBK == GBK - 1 or lb == NBLK - 1:
                    tail0(lb - gi, gi + 1)

        # ---- P3: layer 1 ----
        nc.sync.dma_start(out=idx1_s[:], in_=t_in["idx1s"][:])
        W1R = 2 * (HID + 2)

        def tail1(g0, gn):
            red4 = state["red4"]
            o17 = wp.tile([P, GBK, HID + 2], F32, tag="o17")
            nc.vector.tensor_add(out=o17[:, 0:gn, :],
                                 in0=red4[:, 0:gn, 0, :],
                                 in1=red4[:, 0:gn, 1, :])
            nc.vector.tensor_scalar_add(
                out=_apd(o17[:, 0:1, HID:HID + 1], [[HID + 2, gn], [1, 1]]),
                in0=_apd(o17[:, 0:1, HID:HID + 1], [[HID + 2, gn], [1, 1]]),
                scalar1=1e-16)
            recg = wp.tile([P, GBK], F32, tag="recg")
            nc.vector.reciprocal(
                out=recg[:, 0:gn],
                in_=_apd(o17[:, 0:1, HID:HID + 1], [[HID + 2, gn], [1, 1]]))
            nc.vector.tensor_tensor(
                out=_apd(o17[:, 0:1, 0:1], [[HID + 2, gn], [1, HID]]),
                in0=_apd(o17[:, 0:1, 0:1], [[HID + 2, gn], [1, HID]]),
                in1=_apd(recg[:, 0:1], [[1, gn], [0, HID]]),
                op=ALU.mult)
            nc.vector.tensor_tensor(
                out=_apd(o17[:, 0:1, 0:1], [[HID + 2, gn], [1, HID]]),
                in0=_apd(o17[:, 0:1, 0:1], [[HID + 2, gn], [1, HID]]),
                in1=t1x[:, g0:g0 + gn, 1:HID + 1], op=ALU.add)
            vv = _apd(o17[:, 0:1, 0:1], [[HID + 2, gn], [1, HID]])
            r2c = wp.tile([P, GBK, HID], F32, tag="r2y")
            nc.scalar.activation(r2c[:, 0:gn, :], vv, AF.Relu)
            r1c = wp.tile([P, GBK, HID], F32, tag="r1y")
            nc.scalar.activation(r1c[:, 0:gn, :], vv, AF.Relu, scale=-1.0)
            nc.scalar.activation(r1c[:, 0:gn, :], r1c[:, 0:gn, :],
                                 AF.Exp, scale=-1.0)
            ys = wp.tile([P, GBK, HID], F32, tag="ys")
            nc.vector.tensor_add(out=ys[:, 0:gn, :], in0=r2c[:, 0:gn, :],
                                 in1=r1c[:, 0:gn, :])
            nc.vector.tensor_scalar_add(out=ys[:, 0:gn, :],
                                        in0=ys[:, 0:gn, :], scalar1=-1.0)
            nc.sync.dma_start(
                out=y[g0 * P:(g0 + gn) * P, :].rearrange(
                    "(b p) f -> p b f", p=P),
                in_=ys[:, 0:gn, :])

        for ci, (lb0, lb1) in enumerate(chunks):
            scol, ncol = cum[lb0], cum[lb1] - cum[lb0]
            gt = gp.tile([P, CS, ROW0], BF16, tag="gath")
            g1 = gt[:].rearrange("p s x -> p (s x)").rearrange(
                "p (s x) -> p s x", x=ROW1)
            nc.gpsimd.dma_gather(
                out_ap=g1[:, 0:ncol, :], in_ap=t1v[:, :],
                idxs_ap=idx1_s[:, scol * 8:(scol + ncol) * 8],
                num_idxs=ncol * P, num_idxs_reg=ncol * P,
                elem_size=ROW1, single_packet=False, queue_num=qn(ci))
            al1 = wp.tile([P, CS, 2], BF16, tag="al1")
            nc.vector.tensor_tensor(
                out=al1[:, 0:ncol, :],
                in0=_apd(g1[:, 0:1, HID:HID + 1], [[ROW1, ncol], [AGW, 2]]),
                in1=_apd(adst1_x[:, scol:scol + 1], [[1, ncol], [0, 2]]),
                op=ALU.add)
            avf = al1[:, 0:ncol, :].rearrange("p s t -> p (s t)")
            nc.scalar.activation(avf, avf, AF.Prelu, alpha=0.2,
                                 bias=a1bias_t[:])
            nc.scalar.activation(avf, avf, AF.Exp)
            ws1 = wp.tile([P, CS, 2], BF16, tag="ws1")
            nc.vector.tensor_tensor(
                out=ws1[:, 0:ncol, :], in0=al1[:, 0:ncol, :],
                in1=bsel2b_s[:, scol * 2:(scol + ncol) * 2]
                    .rearrange("p (s t) -> p s t", t=2),
                op=ALU.mult)
            msgt = mp.tile([P, CS, 2 * FH], BF16, tag="msg")
            m1b = msgt[:].rearrange("p s x -> p (s x)")
            nc.vector.tensor_tensor(
                out=_apd(m1b[:, 0:1], [[W1R, ncol], [HID + 2, 2], [1, HID]]),
                in0=_apd(g1[:, 0:1, 0:1],
                         [[ROW1, ncol], [AGW, 2], [1, HID]]),
                in1=_apd(ws1[:, 0:1, 0:1],
                         [[2, ncol], [1, 2], [0, HID]]),
                op=ALU.mult)
            nc.vector.tensor_copy(
                out=_apd(m1b[:, HID:HID + 1],
                         [[W1R, ncol], [HID + 2, 2], [1, 2]]),
                in_=ws1[:, 0:ncol, :].unsqueeze(3)
                    .to_broadcast([P, ncol, 2, 2]))
            for lb in range(lb0, lb1):
                Jb, off = J[lb], cum[lb] - scol
                if lb % GBK == 0:
                    state["red4"] = wp.tile([P, GBK, 2, HID + 2], F32,
                                            tag="red4", name="red4")
                gi = lb % GBK
                nc.vector.tensor_reduce(
                    out=state["red4"][:, gi, :, :].rearrange(
                        "p t f -> p (t f)"),
                    in_=_apd(m1b[:, off * W1R:off * W1R + 1],
                             [[1, W1R], [W1R, Jb]]),
                    axis=mybir.AxisListType.X, op=ALU.add)
                if lb % GBK == GBK - 1 or lb == NBLK - 1:
                    tail1(lb - gi, gi + 1)

    nc.compile()
    return nc


# ======================================================================
from concourse import bass_utils as _bass_utils

_CACHE = {}


def kernel(**inputs):
    x = np.asarray(inputs["x"], np.float32)
    edge_index = np.asarray(inputs["edge_index"])
    cfg = Cfg(N=50000, E=int(edge_index.shape[1]))
    in_maps, new2old = host_prep(cfg, x, edge_index, inputs)
    sig = (tuple(cfg.J), cfg.a1bias)
    if _CACHE.get("sig") != sig:
        _CACHE["nc"] = build_program(cfg)
        _CACHE["sig"] = sig
    nc = _CACHE["nc"]
    res = _bass_utils.run_bass_kernel_spmd(
        nc, in_maps, core_ids=list(range(cfg.NCORES)))
    yfull = np.concatenate([res.results[c]["y"] for c in range(cfg.NCORES)],
                           axis=0)
    out = np.zeros((cfg.N, cfg.HID), np.float32)
    valid = new2old < cfg.N
    out[new2old[valid]] = yfull[valid]
    return out
